# revision 46
# baseline (speedup 1.0000x reference)
"""Multi-head self-attention (RoPE, causal) on 8 trn2 NeuronCores.

Sharding: batch (4) x head-group (2x8 heads) = 8 shards, one per core.
Host sums the two partial o_proj outputs of each batch pair (the
tensor-parallel all-reduce) and concatenates batches.

v3 design: the scalar engine (exp) is the pacing engine in attention,
so everything else is organized to keep it saturated and off its queue.
 - all inputs bf16 (host-converted): halves DMA and SBUF, same PE rate.
 - P0: V projection and Q/K projection for ALL head-pairs, rope via a
   DVE stream_shuffle (rope pairs grouped in 16-row blocks) + DVE
   multiplies + Pool add. ACT only does the V PSUM->SBUF copies.
 - softmax denominators come free from a ones-column appended to V
   (PV matmul emits 65 output rows; row 64 = sum of exp).
 - attention: two head-pairs interleaved, exp batched over both heads
   of a pair ([128, 2, 512] per call), depth-2 software pipeline with
   cross-qb score prologue so ACT never waits; causal mask applied
   after exp as a 0/1 multiply on the Pool engine.
 - softmax normalize: denominator row -> SBUF -> reciprocal_approx_fast
   (DVE; its PSUM-input path is broken on HW), ones-matmul broadcast on
   the PE into a borrowed score slot, DVE multiply into SBUF-resident
   bf16 aT.
 - o_proj tail: PSUM pools rescoped after attention, streams out bf16.
"""
import sys
import math

sys.path.insert(0, "/opt/trn_rl_repo")

import numpy as np
import ml_dtypes
from contextlib import ExitStack

import concourse.bacc as bacc
import concourse.tile as tile
from concourse import mybir
from concourse.bass_utils import run_bass_kernel_spmd

B, S, D, H, DK = 4, 2048, 1024, 16, 64
NCORES = 8
ND = D // 128          # 8 d-tiles of the model dim
NT = S // 512          # 4 token super-blocks
NKT = S // 128         # 16 key/token 128-blocks
HPC = H // 2           # heads per core = 8
NHP = HPC // 2         # head-pairs per core = 4
F32 = mybir.dt.float32
F32R = mybir.dt.float32r
BF16 = mybir.dt.bfloat16
EXPF = mybir.ActivationFunctionType.Exp
COPYF = mybir.ActivationFunctionType.Copy

# stream_shuffle mask: swap 16-row halves within each 32-partition quadrant
SWAP16 = list(range(16, 32)) + list(range(0, 16))

DEBUG = False
_CACHE = {}


def _build():
    nc = bacc.Bacc("TRN2", target_bir_lowering=False, num_devices=NCORES)

    xT_d = nc.dram_tensor("xT", [D, S], BF16, kind="ExternalInput")
    wq_d = nc.dram_tensor("wq", [D, HPC * DK], BF16, kind="ExternalInput")
    wk_d = nc.dram_tensor("wk", [D, HPC * DK], BF16, kind="ExternalInput")
    wv_d = nc.dram_tensor("wv", [D, HPC * DK], BF16, kind="ExternalInput")
    wo_d = nc.dram_tensor("wo", [HPC * DK, D], BF16, kind="ExternalInput")
    ropeC_d = nc.dram_tensor("ropeC", [128, S], BF16, kind="ExternalInput")
    ropeS_d = nc.dram_tensor("ropeS", [128, S], BF16, kind="ExternalInput")
    mask_d = nc.dram_tensor("mask", [128, 128], BF16, kind="ExternalInput")
    yT_d = nc.dram_tensor("yT", [D, S], BF16, kind="ExternalOutput")

    with ExitStack() as ctx:
        tc = ctx.enter_context(tile.TileContext(nc))

        const = ctx.enter_context(tc.tile_pool(name="const", bufs=1))
        xpool = ctx.enter_context(tc.tile_pool(name="x", bufs=1))
        vpool = ctx.enter_context(tc.tile_pool(name="v", bufs=1))
        qkpool = ctx.enter_context(tc.tile_pool(name="qk", bufs=1))
        apool = ctx.enter_context(tc.tile_pool(name="a", bufs=1))
        wopool = ctx.enter_context(tc.tile_pool(name="wo", bufs=1))
        es = ctx.enter_context(tc.tile_pool(name="es", bufs=3))
        tmp = ctx.enter_context(tc.tile_pool(name="tmp", bufs=3))
        ypool = ctx.enter_context(tc.tile_pool(name="y", bufs=2))

        # ---- constants -------------------------------------------------
        ropeC = const.tile([128, S], BF16)
        ropeS = const.tile([128, S], BF16)
        maskt2 = const.tile([128, 2, 128], BF16)
        ones_b = const.tile([1, 64], BF16)
        nc.vector.memset(ones_b, 1.0)

        # ---- persistent tensors ---------------------------------------
        xT = xpool.tile([128, ND, S], BF16)
        # V with a ones column per head: [k, t, head, 65]
        V = vpool.tile([128, NKT, HPC, DK + 1], BF16)
        nc.vector.memset(V[:, :, :, DK : DK + 1], 1.0)
        qk_tiles = []
        for hp in range(NHP):
            qt_t = qkpool.tile([128, S], BF16, tag=f"qt{hp}")
            kt_t = qkpool.tile([128, S], BF16, tag=f"kt{hp}")
            qk_tiles.append((qt_t, kt_t))
        aT = apool.tile([128, NHP, S], BF16)
        wo_sb = wopool.tile([128, NHP, D], BF16)

        # ---- P0: DMAs, V projection, Q/K projection + rope ------------
        with ExitStack() as p0:
            wvpool = p0.enter_context(tc.tile_pool(name="wv", bufs=1))
            stage0 = p0.enter_context(tc.tile_pool(name="st0", bufs=2))
            p0ps = p0.enter_context(
                tc.tile_pool(name="p0ps", bufs=2, space="PSUM")
            )

            wv_sb = wvpool.tile([128, ND, HPC * DK], BF16)
            for d in range(ND):
                nc.sync.dma_start(
                    out=xT[:, d, 0:1024],
                    in_=xT_d[128 * d : 128 * (d + 1), 0:1024],
                )
                nc.sync.dma_start(
                    out=wv_sb[:, d, :], in_=wv_d[128 * d : 128 * (d + 1), :]
                )
            for d in range(ND):
                nc.sync.dma_start(
                    out=xT[:, d, 1024:S],
                    in_=xT_d[128 * d : 128 * (d + 1), 1024:S],
                )

            stages = {}

            def stage_dma(w_d, hp, wtag):
                wt = stage0.tile(
                    [128, ND, 128], BF16, tag=wtag, name=f"w{wtag}{hp}"
                )
                for d in range(ND):
                    nc.gpsimd.dma_start(
                        out=wt[:, d, :],
                        in_=w_d[
                            128 * d : 128 * (d + 1),
                            128 * hp : 128 * (hp + 1),
                        ],
                    )
                stages[(hp, wtag)] = wt

            stage_dma(wq_d, 0, "wq")
            stage_dma(wk_d, 0, "wk")
            nc.gpsimd.dma_start(out=ropeC[:, 0:1024], in_=ropeC_d[:, 0:1024])
            nc.gpsimd.dma_start(out=ropeC[:, 1024:S], in_=ropeC_d[:, 1024:S])
            nc.gpsimd.dma_start(out=ropeS[:, 0:1024], in_=ropeS_d[:, 0:1024])
            nc.gpsimd.dma_start(out=ropeS[:, 1024:S], in_=ropeS_d[:, 1024:S])
            for h2 in range(2):
                nc.gpsimd.dma_start(out=maskt2[:, h2, :], in_=mask_d[:, :])
            stage_dma(wq_d, 1, "wq")
            stage_dma(wk_d, 1, "wk")
            for dd in range(NHP):
                nc.gpsimd.dma_start(
                    out=wo_sb[:, dd, :],
                    in_=wo_d[128 * dd : 128 * (dd + 1), :],
                )

            # V projection: V[t, ev]
            for t in range(NKT):
                psv = p0ps.tile([128, 512], F32, tag="ps", name="psv")
                for d in range(ND):
                    nc.tensor.matmul(
                        psv[:, :],
                        xT[:, d, 128 * t : 128 * (t + 1)],
                        wv_sb[:, d, :],
                        start=(d == 0),
                        stop=(d == ND - 1),
                    )
                nc.scalar.activation(
                    V[:, t, :, 0:DK],
                    psv[:, :].rearrange("p (h d) -> p h d", h=HPC),
                    COPYF,
                )

            # Q/K projection for all head-pairs
            def proj_unit(hp, wtag, OUT, tb):
                cols = slice(512 * tb, 512 * (tb + 1))
                psq = p0ps.tile([128, 512], F32, tag="ps", name="psq")
                wt = stages[(hp, wtag)]
                for d in range(ND):
                    nc.tensor.matmul(
                        psq[:, :],
                        wt[:, d, :],
                        xT[:, d, cols],
                        start=(d == 0),
                        stop=(d == ND - 1),
                    )
                q16 = tmp.tile([128, 512], BF16, tag="q16")
                nc.scalar.activation(q16[:, :], psq[:, :], COPYF)
                tsw = tmp.tile([128, 512], BF16, tag="tsw")
                nc.vector.stream_shuffle(tsw[:, :], q16[:, :], SWAP16)
                t1 = tmp.tile([128, 512], BF16, tag="t1")
                nc.vector.tensor_mul(t1[:, :], q16[:, :], ropeC[:, cols])
                t2 = tmp.tile([128, 512], BF16, tag="t2")
                nc.vector.tensor_mul(t2[:, :], tsw[:, :], ropeS[:, cols])
                nc.vector.tensor_add(OUT[:, cols], t1[:, :], t2[:, :])

            for hp in range(NHP):
                if hp == 2:
                    stage_dma(wq_d, 2, "wq")
                    stage_dma(wk_d, 2, "wk")
                if hp == 3:
                    stage_dma(wq_d, 3, "wq")
                    stage_dma(wk_d, 3, "wk")
                QT, KT = qk_tiles[hp]
                for wtag, OUT in (("wq", QT), ("wk", KT)):
                    for tb in range(NT):
                        proj_unit(hp, wtag, OUT, tb)

            if DEBUG:
                nc.sync.dma_start(
                    out=dV_d[:, :],
                    in_=V[:, :, :, :].rearrange("p a b c -> p (a b c)"),
                )

        # ---- attention phases -----------------------------------------
        with ExitStack() as pa:
            pscore = pa.enter_context(
                tc.tile_pool(name="pscore", bufs=2, space="PSUM")
            )
            ppo = pa.enter_context(
                tc.tile_pool(name="ppo", bufs=1, space="PSUM")
            )

            def emit_scores(slot, hp, k, qb):
                """Scores both heads -> one batched exp -> Pool mask."""
                QT, KT = qk_tiles[hp]
                r = k - 4 * qb
                q0 = 128 * r if r >= 0 else 0
                qlo = 512 * qb + q0
                qhi = 512 * (qb + 1)
                pss = pscore.tile([128, 2, 512], F32, tag="s2", name="pss")
                for h2 in range(2):
                    b0 = 64 * h2
                    nc.tensor.matmul(
                        pss[:, h2, q0:512],
                        KT[b0 : b0 + 64, 128 * k : 128 * (k + 1)],
                        QT[b0 : b0 + 64, qlo:qhi],
                        start=True,
                        stop=True,
                        tile_position=(b0, 0),
                        skip_group_check=True,
                    )
                es_t = es.tile(
                    [128, 2, 512], BF16, tag=f"es{slot}", name="es_t"
                )
                nc.scalar.activation(
                    es_t[:, :, q0:512], pss[:, :, q0:512], EXPF
                )
                if r >= 0:
                    nc.gpsimd.tensor_mul(
                        es_t[:, :, q0 : q0 + 128],
                        es_t[:, :, q0 : q0 + 128],
                        maskt2[:, :, :],
                    )
                return es_t

            def emit_pv(hp, po, es_t, k, qb, nkb):
                r = k - 4 * qb
                q0 = 128 * r if r >= 0 else 0
                for h2 in range(2):
                    nc.tensor.matmul(
                        po[0:65, h2, q0:512],
                        V[:, k, 2 * hp + h2, :],
                        es_t[:, h2, q0:512],
                        start=(k == 0),
                        stop=(k == nkb - 1),
                        skip_group_check=True,
                    )

            def normalize_pre(tagx, po):
                den_sb = tmp.tile([1, 2, 512], F32, tag=f"den{tagx}", bufs=1)
                nc.vector.tensor_copy(den_sb[0:1, :, :], po[64:65, :, :])
                rec = tmp.tile([1, 2, 512], F32, tag=f"rec{tagx}", bufs=1)
                nc.vector.reciprocal_approx_fast(
                    rec[0:1, :, :], den_sb[0:1, :, :]
                )
                rec_b = tmp.tile([1, 2, 512], BF16, tag=f"recb{tagx}", bufs=1)
                nc.vector.tensor_copy(rec_b[0:1, :, :], rec[0:1, :, :])
                return rec_b

            def normalize_post(tagx, hp, po, qb, rec_b):
                qcols = slice(512 * qb, 512 * (qb + 1))
                psb = pscore.tile([128, 512], F32, tag="s2", name="psb")
                for h2 in range(2):
                    nc.tensor.matmul(
                        psb[64 * h2 : 64 * h2 + 64, :],
                        ones_b[0:1, :],
                        rec_b[0:1, h2, :],
                        start=True,
                        stop=True,
                        tile_position=(0, 64 * h2),
                        skip_group_check=True,
                    )
                recbc = tmp.tile([128, 512], F32, tag=f"recbc{tagx}", bufs=1)
                nc.vector.tensor_copy(recbc[:, :], psb[:, :])
                for h2 in range(2):
                    nc.vector.tensor_mul(
                        aT[64 * h2 : 64 * h2 + 64, hp, qcols],
                        po[0:64, h2, :],
                        recbc[64 * h2 : 64 * h2 + 64, :],
                    )

            def normalize_pair(hpA, hpB, poA, poB, qb):
                rbA = normalize_pre("A", poA)
                rbB = normalize_pre("B", poB)
                normalize_post("A", hpA, poA, qb, rbA)
                normalize_post("B", hpB, poB, qb, rbB)

            def attention_pair(hpA, hpB, pre_final_hook=None):
                # flat item list (qb, k); scores emitted 2 items ahead
                items = []
                for qb in range(NT):
                    nkb = 4 * qb + 4
                    for k in range(nkb):
                        items.append((qb, k, nkb))
                es_cur = {}

                def emit_S(i):
                    if i < len(items):
                        qb, k, _ = items[i]
                        es_cur[("A", i)] = emit_scores("A", hpA, k, qb)
                        es_cur[("B", i)] = emit_scores("B", hpB, k, qb)

                po_t = {}

                def new_po(qb):
                    po_t["A"] = ppo.tile(
                        [65, 2, 512], F32, tag="poA", name="poA"
                    )
                    po_t["B"] = ppo.tile(
                        [65, 2, 512], F32, tag="poB", name="poB"
                    )

                emit_S(0)
                emit_S(1)
                for i, (qb, k, nkb) in enumerate(items):
                    if k == 0:
                        new_po(qb)
                    emit_S(i + 2)
                    emit_pv(hpA, po_t["A"], es_cur.pop(("A", i)), k, qb, nkb)
                    emit_pv(hpB, po_t["B"], es_cur.pop(("B", i)), k, qb, nkb)
                    if k == nkb - 1:
                        if i == len(items) - 1 and pre_final_hook is not None:
                            pre_final_hook()
                        normalize_pair(hpA, hpB, po_t["A"], po_t["B"], qb)

            def oproj_unit(et, tb, psy_pool, pstag, bufs=None):
                kw = {"bufs": bufs} if bufs else {}
                psy = psy_pool.tile(
                    [128, 512], F32, tag=pstag, name="psy", **kw
                )
                for dd in range(NHP):
                    nc.tensor.matmul(
                        psy[:, :],
                        wo_sb[:, dd, 128 * et : 128 * (et + 1)],
                        aT[:, dd, 512 * tb : 512 * (tb + 1)],
                        start=(dd == 0),
                        stop=(dd == NHP - 1),
                    )
                y_t = ypool.tile([128, 512], BF16, tag="y", bufs=6)
                if et % 2 == 0:
                    nc.vector.tensor_copy(y_t[:, :], psy[:, :])
                else:
                    nc.scalar.activation(y_t[:, :], psy[:, :], COPYF)
                for half in range(2):
                    nc.sync.dma_start(
                        out=yT_d[
                            128 * et : 128 * (et + 1),
                            512 * tb + 256 * half : 512 * tb + 256 * (half + 1),
                        ],
                        in_=y_t[:, 256 * half : 256 * (half + 1)],
                    )

            def early_oproj():
                for tb in range(NT - 1):
                    for et in range(ND):
                        oproj_unit(et, tb, pscore, "s2")

            attention_pair(0, 1)
            attention_pair(2, 3, pre_final_hook=early_oproj)

            # o_proj tail: last token block (aT for it just completed)
            for et in range(ND):
                oproj_unit(et, NT - 1, pscore, "s2")

    nc.compile()
    return nc


# host-side prep ------------------------------------------------------------

# per-head row permutation grouping rope pairs in 16-row blocks:
# [evens(f0..15) | odds(f0..15) | evens(f16..31) | odds(f16..31)]
_PERM16 = np.concatenate(
    [
        np.arange(0, 32, 2),
        np.arange(1, 32, 2),
        np.arange(32, 64, 2),
        np.arange(33, 64, 2),
    ]
)


def _rope_tables():
    pos = np.arange(S, dtype=np.float32)
    inv = (10000.0 ** (-(np.arange(0, DK, 2, dtype=np.float32)) / DK)).astype(
        np.float32
    )  # 32 freqs
    ang = pos[None, :] * inv[:, None]  # [32, S]
    c = np.cos(ang).astype(np.float32)
    s = np.sin(ang).astype(np.float32)
    # per head (64 rows): [c(f0-15); c(f0-15); c(f16-31); c(f16-31)]
    C64 = np.concatenate([c[0:16], c[0:16], c[16:32], c[16:32]], axis=0)
    S64 = np.concatenate([-s[0:16], s[0:16], -s[16:32], s[16:32]], axis=0)
    ropeC = np.ascontiguousarray(np.concatenate([C64, C64], axis=0)).astype(
        ml_dtypes.bfloat16
    )
    ropeS = np.ascontiguousarray(np.concatenate([S64, S64], axis=0)).astype(
        ml_dtypes.bfloat16
    )
    return ropeC, ropeS


_ROPEC, _ROPES = _rope_tables()

ki = np.arange(128)[:, None]
qi = np.arange(128)[None, :]
_TRIMASK = np.where(ki <= qi, 1.0, 0.0).astype(ml_dtypes.bfloat16)


def _prep_core_inputs(x, token_positions, w_qkv, w_o, core):
    b = core // 2
    h0 = HPC * (core % 2)

    xT = np.ascontiguousarray(x[b].T).astype(ml_dtypes.bfloat16)

    w_q = w_qkv[0 * D : 1 * D]
    w_k = w_qkv[1 * D : 2 * D]
    w_v = w_qkv[2 * D : 3 * D]

    def gather(w, permute, scale):
        rows = []
        for j in range(HPC):
            g = h0 + j
            blk = w[DK * g : DK * (g + 1)]
            if permute:
                blk = blk[_PERM16]
            rows.append(blk)
        out = np.concatenate(rows, axis=0).astype(np.float32) * scale
        return np.ascontiguousarray(out.T).astype(ml_dtypes.bfloat16)

    wq = gather(w_q, True, 1.0 / math.sqrt(DK))
    wk = gather(w_k, True, 1.0)
    wv = gather(w_v, False, 1.0)

    rows = []
    for j in range(HPC):
        g = h0 + j
        rows.append(w_o[:, DK * g : DK * (g + 1)].T)
    wo = np.ascontiguousarray(np.concatenate(rows, axis=0)).astype(
        ml_dtypes.bfloat16
    )

    return {
        "xT": xT,
        "wq": wq,
        "wk": wk,
        "wv": wv,
        "wo": wo,
        "ropeC": _ROPEC,
        "ropeS": _ROPES,
        "mask": _TRIMASK,
    }


def kernel(x, token_positions, w_qkv, w_o):
    x = np.asarray(x, dtype=np.float32)
    token_positions = np.asarray(token_positions)
    w_qkv = np.asarray(w_qkv, dtype=np.float32)
    w_o = np.asarray(w_o, dtype=np.float32)

    if "nc" not in _CACHE:
        _CACHE["nc"] = _build()
    nc = _CACHE["nc"]

    in_maps = [
        _prep_core_inputs(x, token_positions, w_qkv, w_o, c)
        for c in range(NCORES)
    ]
    res = run_bass_kernel_spmd(nc, in_maps, core_ids=list(range(NCORES)))
    _CACHE["last_results"] = res

    out = np.empty((B, S, D), dtype=np.float32)
    for b in range(B):
        yT = res.results[2 * b]["yT"].astype(np.float32) + res.results[
            2 * b + 1
        ]["yT"].astype(np.float32)
        out[b] = yT.T
    return out


# revision 49
# speedup vs baseline: 1.0102x; 1.0102x over previous
"""Multi-head self-attention (RoPE, causal) on 8 trn2 NeuronCores.

Sharding: batch (4) x head-group (2x8 heads) = 8 shards, one per core.
Host sums the two partial o_proj outputs of each batch pair (the
tensor-parallel all-reduce) and concatenates batches.

v3 design: the scalar engine (exp) is the pacing engine in attention,
so everything else is organized to keep it saturated and off its queue.
 - all inputs bf16 (host-converted): halves DMA and SBUF, same PE rate.
 - P0: V projection and Q/K projection for ALL head-pairs, rope via a
   DVE stream_shuffle (rope pairs grouped in 16-row blocks) + DVE
   multiplies + Pool add. ACT only does the V PSUM->SBUF copies.
 - softmax denominators come free from a ones-column appended to V
   (PV matmul emits 65 output rows; row 64 = sum of exp).
 - attention: two head-pairs interleaved, exp batched over both heads
   of a pair ([128, 2, 512] per call), depth-2 software pipeline with
   cross-qb score prologue so ACT never waits; causal mask applied
   after exp as a 0/1 multiply on the Pool engine.
 - softmax normalize: denominator row -> SBUF -> reciprocal_approx_fast
   (DVE; its PSUM-input path is broken on HW), ones-matmul broadcast on
   the PE into a borrowed score slot, DVE multiply into SBUF-resident
   bf16 aT.
 - o_proj tail: PSUM pools rescoped after attention, streams out bf16.
"""
import sys
import math

sys.path.insert(0, "/opt/trn_rl_repo")

import numpy as np
import ml_dtypes
from contextlib import ExitStack

import concourse.bacc as bacc
import concourse.tile as tile
from concourse import mybir
from concourse.bass_utils import run_bass_kernel_spmd

B, S, D, H, DK = 4, 2048, 1024, 16, 64
NCORES = 8
ND = D // 128          # 8 d-tiles of the model dim
NT = S // 512          # 4 token super-blocks
NKT = S // 128         # 16 key/token 128-blocks
HPC = H // 2           # heads per core = 8
NHP = HPC // 2         # head-pairs per core = 4
F32 = mybir.dt.float32
F32R = mybir.dt.float32r
BF16 = mybir.dt.bfloat16
EXPF = mybir.ActivationFunctionType.Exp
COPYF = mybir.ActivationFunctionType.Copy

# stream_shuffle mask: swap 16-row halves within each 32-partition quadrant
SWAP16 = list(range(16, 32)) + list(range(0, 16))

DEBUG = False
_CACHE = {}


def _build():
    nc = bacc.Bacc("TRN2", target_bir_lowering=False, num_devices=NCORES)

    xT_d = nc.dram_tensor("xT", [D, S], BF16, kind="ExternalInput")
    wq_d = nc.dram_tensor("wq", [D, HPC * DK], BF16, kind="ExternalInput")
    wk_d = nc.dram_tensor("wk", [D, HPC * DK], BF16, kind="ExternalInput")
    wv_d = nc.dram_tensor("wv", [D, HPC * DK], BF16, kind="ExternalInput")
    wo_d = nc.dram_tensor("wo", [HPC * DK, D], BF16, kind="ExternalInput")
    ropeC_d = nc.dram_tensor("ropeC", [128, S], BF16, kind="ExternalInput")
    ropeS_d = nc.dram_tensor("ropeS", [128, S], BF16, kind="ExternalInput")
    mask_d = nc.dram_tensor("mask", [128, 128], BF16, kind="ExternalInput")
    yT_d = nc.dram_tensor("yT", [D, S], BF16, kind="ExternalOutput")

    with ExitStack() as ctx:
        tc = ctx.enter_context(tile.TileContext(nc))

        const = ctx.enter_context(tc.tile_pool(name="const", bufs=1))
        xpool = ctx.enter_context(tc.tile_pool(name="x", bufs=1))
        vpool = ctx.enter_context(tc.tile_pool(name="v", bufs=1))
        qkpool = ctx.enter_context(tc.tile_pool(name="qk", bufs=1))
        apool = ctx.enter_context(tc.tile_pool(name="a", bufs=1))
        wopool = ctx.enter_context(tc.tile_pool(name="wo", bufs=1))
        es = ctx.enter_context(tc.tile_pool(name="es", bufs=3))
        tmp = ctx.enter_context(tc.tile_pool(name="tmp", bufs=3))
        ypool = ctx.enter_context(tc.tile_pool(name="y", bufs=2))

        # ---- constants -------------------------------------------------
        ropeC = const.tile([128, S], BF16)
        ropeS = const.tile([128, S], BF16)
        maskt2 = const.tile([128, 2, 128], BF16)
        ones_b = const.tile([1, 64], BF16)
        nc.vector.memset(ones_b, 1.0)

        # ---- persistent tensors ---------------------------------------
        xT = xpool.tile([128, ND, S], BF16)
        # V with a ones column per head: [k, t, head, 65]
        V = vpool.tile([128, NKT, HPC, DK + 1], BF16)
        nc.vector.memset(V[:, :, :, DK : DK + 1], 1.0)
        qk_tiles = []
        for hp in range(NHP):
            qt_t = qkpool.tile([128, S], BF16, tag=f"qt{hp}")
            kt_t = qkpool.tile([128, S], BF16, tag=f"kt{hp}")
            qk_tiles.append((qt_t, kt_t))
        aT = apool.tile([128, NHP, S], BF16)
        wo_sb = wopool.tile([128, NHP, D], BF16)

        # ---- P0: DMAs, V projection, Q/K projection + rope ------------
        with ExitStack() as p0:
            wvpool = p0.enter_context(tc.tile_pool(name="wv", bufs=1))
            stage0 = p0.enter_context(tc.tile_pool(name="st0", bufs=2))
            p0ps = p0.enter_context(
                tc.tile_pool(name="p0ps", bufs=2, space="PSUM")
            )

            wv_sb = wvpool.tile([128, ND, HPC * DK], BF16)
            for d in range(ND):
                nc.sync.dma_start(
                    out=xT[:, d, 0:1024],
                    in_=xT_d[128 * d : 128 * (d + 1), 0:1024],
                )
                nc.gpsimd.dma_start(
                    out=wv_sb[:, d, :], in_=wv_d[128 * d : 128 * (d + 1), :]
                )
            for d in range(ND):
                nc.sync.dma_start(
                    out=xT[:, d, 1024:S],
                    in_=xT_d[128 * d : 128 * (d + 1), 1024:S],
                )

            stages = {}

            def stage_dma(w_d, hp, wtag):
                wt = stage0.tile(
                    [128, ND, 128], BF16, tag=wtag, name=f"w{wtag}{hp}"
                )
                for d in range(ND):
                    nc.gpsimd.dma_start(
                        out=wt[:, d, :],
                        in_=w_d[
                            128 * d : 128 * (d + 1),
                            128 * hp : 128 * (hp + 1),
                        ],
                    )
                stages[(hp, wtag)] = wt

            stage_dma(wq_d, 0, "wq")
            stage_dma(wk_d, 0, "wk")
            nc.gpsimd.dma_start(out=ropeC[:, 0:1024], in_=ropeC_d[:, 0:1024])
            nc.gpsimd.dma_start(out=ropeC[:, 1024:S], in_=ropeC_d[:, 1024:S])
            nc.gpsimd.dma_start(out=ropeS[:, 0:1024], in_=ropeS_d[:, 0:1024])
            nc.gpsimd.dma_start(out=ropeS[:, 1024:S], in_=ropeS_d[:, 1024:S])
            for h2 in range(2):
                nc.gpsimd.dma_start(out=maskt2[:, h2, :], in_=mask_d[:, :])
            stage_dma(wq_d, 1, "wq")
            stage_dma(wk_d, 1, "wk")
            for dd in range(NHP):
                nc.gpsimd.dma_start(
                    out=wo_sb[:, dd, :],
                    in_=wo_d[128 * dd : 128 * (dd + 1), :],
                )

            # V projection: V[t, ev]
            for t in range(NKT):
                psv = p0ps.tile([128, 512], F32, tag="ps", name="psv")
                for d in range(ND):
                    nc.tensor.matmul(
                        psv[:, :],
                        xT[:, d, 128 * t : 128 * (t + 1)],
                        wv_sb[:, d, :],
                        start=(d == 0),
                        stop=(d == ND - 1),
                    )
                nc.scalar.activation(
                    V[:, t, :, 0:DK],
                    psv[:, :].rearrange("p (h d) -> p h d", h=HPC),
                    COPYF,
                )

            # Q/K projection for all head-pairs
            def proj_unit(hp, wtag, OUT, tb):
                cols = slice(512 * tb, 512 * (tb + 1))
                psq = p0ps.tile([128, 512], F32, tag="ps", name="psq")
                wt = stages[(hp, wtag)]
                for d in range(ND):
                    nc.tensor.matmul(
                        psq[:, :],
                        wt[:, d, :],
                        xT[:, d, cols],
                        start=(d == 0),
                        stop=(d == ND - 1),
                    )
                q16 = tmp.tile([128, 512], BF16, tag="q16")
                nc.scalar.activation(q16[:, :], psq[:, :], COPYF)
                tsw = tmp.tile([128, 512], BF16, tag="tsw")
                nc.vector.stream_shuffle(tsw[:, :], q16[:, :], SWAP16)
                t1 = tmp.tile([128, 512], BF16, tag="t1")
                nc.vector.tensor_mul(t1[:, :], q16[:, :], ropeC[:, cols])
                t2 = tmp.tile([128, 512], BF16, tag="t2")
                nc.vector.tensor_mul(t2[:, :], tsw[:, :], ropeS[:, cols])
                nc.vector.tensor_add(OUT[:, cols], t1[:, :], t2[:, :])

            for hp in range(NHP):
                if hp == 2:
                    stage_dma(wq_d, 2, "wq")
                    stage_dma(wk_d, 2, "wk")
                if hp == 3:
                    stage_dma(wq_d, 3, "wq")
                    stage_dma(wk_d, 3, "wk")
                QT, KT = qk_tiles[hp]
                for wtag, OUT in (("wq", QT), ("wk", KT)):
                    for tb in range(NT):
                        proj_unit(hp, wtag, OUT, tb)

            if DEBUG:
                nc.sync.dma_start(
                    out=dV_d[:, :],
                    in_=V[:, :, :, :].rearrange("p a b c -> p (a b c)"),
                )

        # ---- attention phases -----------------------------------------
        with ExitStack() as pa:
            pscore = pa.enter_context(
                tc.tile_pool(name="pscore", bufs=2, space="PSUM")
            )
            ppo = pa.enter_context(
                tc.tile_pool(name="ppo", bufs=1, space="PSUM")
            )

            def emit_scores(slot, hp, k, qb):
                """Scores both heads -> one batched exp -> Pool mask."""
                QT, KT = qk_tiles[hp]
                r = k - 4 * qb
                q0 = 128 * r if r >= 0 else 0
                qlo = 512 * qb + q0
                qhi = 512 * (qb + 1)
                pss = pscore.tile([128, 2, 512], F32, tag="s2", name="pss")
                for h2 in range(2):
                    b0 = 64 * h2
                    nc.tensor.matmul(
                        pss[:, h2, q0:512],
                        KT[b0 : b0 + 64, 128 * k : 128 * (k + 1)],
                        QT[b0 : b0 + 64, qlo:qhi],
                        start=True,
                        stop=True,
                        tile_position=(b0, 0),
                        skip_group_check=True,
                    )
                es_t = es.tile(
                    [128, 2, 512], BF16, tag=f"es{slot}", name="es_t"
                )
                nc.scalar.activation(
                    es_t[:, :, q0:512], pss[:, :, q0:512], EXPF
                )
                if r >= 0:
                    nc.gpsimd.tensor_mul(
                        es_t[:, :, q0 : q0 + 128],
                        es_t[:, :, q0 : q0 + 128],
                        maskt2[:, :, :],
                    )
                return es_t

            def emit_pv(hp, po, es_t, k, qb, nkb):
                r = k - 4 * qb
                q0 = 128 * r if r >= 0 else 0
                for h2 in range(2):
                    nc.tensor.matmul(
                        po[0:65, h2, q0:512],
                        V[:, k, 2 * hp + h2, :],
                        es_t[:, h2, q0:512],
                        start=(k == 0),
                        stop=(k == nkb - 1),
                        skip_group_check=True,
                    )

            def normalize_pre(tagx, po):
                den_sb = tmp.tile([1, 2, 512], F32, tag=f"den{tagx}", bufs=1)
                nc.vector.tensor_copy(den_sb[0:1, :, :], po[64:65, :, :])
                rec = tmp.tile([1, 2, 512], F32, tag=f"rec{tagx}", bufs=1)
                nc.vector.reciprocal_approx_fast(
                    rec[0:1, :, :], den_sb[0:1, :, :]
                )
                rec_b = tmp.tile([1, 2, 512], BF16, tag=f"recb{tagx}", bufs=1)
                nc.vector.tensor_copy(rec_b[0:1, :, :], rec[0:1, :, :])
                return rec_b

            def normalize_post(tagx, hp, po, qb, rec_b):
                qcols = slice(512 * qb, 512 * (qb + 1))
                psb = pscore.tile([128, 512], F32, tag="s2", name="psb")
                for h2 in range(2):
                    nc.tensor.matmul(
                        psb[64 * h2 : 64 * h2 + 64, :],
                        ones_b[0:1, :],
                        rec_b[0:1, h2, :],
                        start=True,
                        stop=True,
                        tile_position=(0, 64 * h2),
                        skip_group_check=True,
                    )
                recbc = tmp.tile([128, 512], F32, tag=f"recbc{tagx}", bufs=1)
                nc.vector.tensor_copy(recbc[:, :], psb[:, :])
                for h2 in range(2):
                    nc.vector.tensor_mul(
                        aT[64 * h2 : 64 * h2 + 64, hp, qcols],
                        po[0:64, h2, :],
                        recbc[64 * h2 : 64 * h2 + 64, :],
                    )

            def normalize_pair(hpA, hpB, poA, poB, qb):
                rbA = normalize_pre("A", poA)
                rbB = normalize_pre("B", poB)
                normalize_post("A", hpA, poA, qb, rbA)
                normalize_post("B", hpB, poB, qb, rbB)

            def attention_pair(hpA, hpB, pre_final_hook=None):
                # flat item list (qb, k); scores emitted 2 items ahead
                items = []
                for qb in range(NT):
                    nkb = 4 * qb + 4
                    for k in range(nkb):
                        items.append((qb, k, nkb))
                es_cur = {}

                def emit_S(i):
                    if i < len(items):
                        qb, k, _ = items[i]
                        es_cur[("A", i)] = emit_scores("A", hpA, k, qb)
                        es_cur[("B", i)] = emit_scores("B", hpB, k, qb)

                po_t = {}

                def new_po(qb):
                    po_t["A"] = ppo.tile(
                        [65, 2, 512], F32, tag="poA", name="poA"
                    )
                    po_t["B"] = ppo.tile(
                        [65, 2, 512], F32, tag="poB", name="poB"
                    )

                emit_S(0)
                emit_S(1)
                pending_norm = [None]
                for i, (qb, k, nkb) in enumerate(items):
                    if k == 0:
                        new_po(qb)
                    emit_S(i + 2)
                    if pending_norm[0] is not None:
                        pA, pB, pqb = pending_norm[0]
                        pending_norm[0] = None
                        normalize_pair(hpA, hpB, pA, pB, pqb)
                    emit_pv(hpA, po_t["A"], es_cur.pop(("A", i)), k, qb, nkb)
                    emit_pv(hpB, po_t["B"], es_cur.pop(("B", i)), k, qb, nkb)
                    if k == nkb - 1:
                        if i == len(items) - 1:
                            if pre_final_hook is not None:
                                pre_final_hook()
                            normalize_pair(
                                hpA, hpB, po_t["A"], po_t["B"], qb
                            )
                        else:
                            pending_norm[0] = (po_t["A"], po_t["B"], qb)

            def oproj_unit(et, tb, psy_pool, pstag, bufs=None):
                kw = {"bufs": bufs} if bufs else {}
                psy = psy_pool.tile(
                    [128, 512], F32, tag=pstag, name="psy", **kw
                )
                for dd in range(NHP):
                    nc.tensor.matmul(
                        psy[:, :],
                        wo_sb[:, dd, 128 * et : 128 * (et + 1)],
                        aT[:, dd, 512 * tb : 512 * (tb + 1)],
                        start=(dd == 0),
                        stop=(dd == NHP - 1),
                    )
                y_t = ypool.tile([128, 512], BF16, tag="y", bufs=6)
                if et % 2 == 0:
                    nc.vector.tensor_copy(y_t[:, :], psy[:, :])
                else:
                    nc.scalar.activation(y_t[:, :], psy[:, :], COPYF)
                eng = nc.sync if et % 2 == 0 else nc.gpsimd
                eng.dma_start(
                    out=yT_d[
                        128 * et : 128 * (et + 1),
                        512 * tb : 512 * (tb + 1),
                    ],
                    in_=y_t[:, :],
                )

            def early_oproj():
                for tb in range(NT - 1):
                    for et in range(ND):
                        oproj_unit(et, tb, pscore, "s2")

            attention_pair(0, 1)
            attention_pair(2, 3, pre_final_hook=early_oproj)

            # o_proj tail: last token block (aT for it just completed)
            for et in range(ND):
                oproj_unit(et, NT - 1, pscore, "s2")

    nc.compile()
    return nc


# host-side prep ------------------------------------------------------------

# per-head row permutation grouping rope pairs in 16-row blocks:
# [evens(f0..15) | odds(f0..15) | evens(f16..31) | odds(f16..31)]
_PERM16 = np.concatenate(
    [
        np.arange(0, 32, 2),
        np.arange(1, 32, 2),
        np.arange(32, 64, 2),
        np.arange(33, 64, 2),
    ]
)


def _rope_tables():
    pos = np.arange(S, dtype=np.float32)
    inv = (10000.0 ** (-(np.arange(0, DK, 2, dtype=np.float32)) / DK)).astype(
        np.float32
    )  # 32 freqs
    ang = pos[None, :] * inv[:, None]  # [32, S]
    c = np.cos(ang).astype(np.float32)
    s = np.sin(ang).astype(np.float32)
    # per head (64 rows): [c(f0-15); c(f0-15); c(f16-31); c(f16-31)]
    C64 = np.concatenate([c[0:16], c[0:16], c[16:32], c[16:32]], axis=0)
    S64 = np.concatenate([-s[0:16], s[0:16], -s[16:32], s[16:32]], axis=0)
    ropeC = np.ascontiguousarray(np.concatenate([C64, C64], axis=0)).astype(
        ml_dtypes.bfloat16
    )
    ropeS = np.ascontiguousarray(np.concatenate([S64, S64], axis=0)).astype(
        ml_dtypes.bfloat16
    )
    return ropeC, ropeS


_ROPEC, _ROPES = _rope_tables()

ki = np.arange(128)[:, None]
qi = np.arange(128)[None, :]
_TRIMASK = np.where(ki <= qi, 1.0, 0.0).astype(ml_dtypes.bfloat16)


def _prep_core_inputs(x, token_positions, w_qkv, w_o, core):
    b = core // 2
    h0 = HPC * (core % 2)

    xT = np.ascontiguousarray(x[b].T).astype(ml_dtypes.bfloat16)

    w_q = w_qkv[0 * D : 1 * D]
    w_k = w_qkv[1 * D : 2 * D]
    w_v = w_qkv[2 * D : 3 * D]

    def gather(w, permute, scale):
        rows = []
        for j in range(HPC):
            g = h0 + j
            blk = w[DK * g : DK * (g + 1)]
            if permute:
                blk = blk[_PERM16]
            rows.append(blk)
        out = np.concatenate(rows, axis=0).astype(np.float32) * scale
        return np.ascontiguousarray(out.T).astype(ml_dtypes.bfloat16)

    wq = gather(w_q, True, 1.0 / math.sqrt(DK))
    wk = gather(w_k, True, 1.0)
    wv = gather(w_v, False, 1.0)

    rows = []
    for j in range(HPC):
        g = h0 + j
        rows.append(w_o[:, DK * g : DK * (g + 1)].T)
    wo = np.ascontiguousarray(np.concatenate(rows, axis=0)).astype(
        ml_dtypes.bfloat16
    )

    return {
        "xT": xT,
        "wq": wq,
        "wk": wk,
        "wv": wv,
        "wo": wo,
        "ropeC": _ROPEC,
        "ropeS": _ROPES,
        "mask": _TRIMASK,
    }


def kernel(x, token_positions, w_qkv, w_o):
    x = np.asarray(x, dtype=np.float32)
    token_positions = np.asarray(token_positions)
    w_qkv = np.asarray(w_qkv, dtype=np.float32)
    w_o = np.asarray(w_o, dtype=np.float32)

    if "nc" not in _CACHE:
        _CACHE["nc"] = _build()
    nc = _CACHE["nc"]

    in_maps = [
        _prep_core_inputs(x, token_positions, w_qkv, w_o, c)
        for c in range(NCORES)
    ]
    res = run_bass_kernel_spmd(nc, in_maps, core_ids=list(range(NCORES)))
    _CACHE["last_results"] = res

    out = np.empty((B, S, D), dtype=np.float32)
    for b in range(B):
        yT = res.results[2 * b]["yT"].astype(np.float32) + res.results[
            2 * b + 1
        ]["yT"].astype(np.float32)
        out[b] = yT.T
    return out
